# revision 68
# baseline (speedup 1.0000x reference)
"""Trainium2 Bass kernel: batched dense attention (v4).

Full inputs: queries/keys/values [16, 2048, 64] fp32.
Shards batch dim across 8 NeuronCores (2 batches per core).

Per-core pipeline (batches A=0, B=1 local):
  HBM loads use the "(p t)" row split (row n = p*16 + t) so every DMA
  descriptor moves >=1KB contiguous (no <512B 2x latency penalty).
  Q/K: HBM fp32 -> staged SBUF -> cast fp16 (interleaved [q, t, (b,d)])
  -> transpose into qt/kt [(b*64+d), t, q].  K (needed tile-by-tile from
  the first groups) via PE transposes drained by DVE/ACT; Q tiles 4:16
  (lots of slack) via XBAR DMA transpose, keeping drain engines free.
  S^T[j, q] = K . Q    (PE fp16, fp32 PSUM, 1-2 slot groups, 6-bank ring)
  P^T = exp(S^T / 8) in fp16; WHOLE drain groups alternate between:
     ACT: exact exp (scale=0.125 fused), 2-slot groups (amortizes its
          185ns PSUM/SBUF access overhead over 1024 elements)
     DVE: Schraudolph bit-trick exp (int16 mult+add == fp16 exp bits),
          1-slot groups
  via a greedy offline plan that levels both engines' modeled busy time.
  O[q, d] = sum_j P^T[j, q] V'[j, d]  (PE: P^T stationary, V'=[V|1]
  moving; 65-col matmuls; softmax sums ride the ones column)
  out = O[:, 0:64] * (1 / O[:, 64])   (DVE reciprocal + mult from PSUM)

Scheduling notes (TimelineSim cost model):
  - The pipeline is PE-throughput-bound (~321 ns/slot: 213 QK + 108 PV);
    drain engines run ~94% of PE busy.  Dummy matmuls pre-warm the PE
    p-state ramp (full clock needs ~3us of activity) during loads.
  - Every DMA rides the single SP HWDGE queue in FIFO order; cross-queue
    DMA pairs get sem-chained by the tile scheduler (~1.5us per switch).
  - The last q-block runs batch-major so only batch 1's drain -> PV ->
    normalize -> store chain (~4.2us) trails the final drain.
  - fp8e4m3 DoubleRow QK was tried and rejected: Q/K quantization costs
    ~10% output error (softmax amplifies score noise) vs the 2e-2 budget.
  - PSUM: 2x2-bank ACT ring + 2x1-bank DVE ring + 2 PV banks = 8 banks.
"""

import sys
for _p in ("/opt/trn_rl_repo", "/root/.axon_site/_ro/trn_rl_repo"):
    if _p not in sys.path:
        sys.path.insert(0, _p)

import math
import numpy as np

import concourse.bass as bass
import concourse.mybir as mybir
from concourse import bacc
from concourse.tile import TileContext
from concourse.masks import make_identity

F32 = mybir.dt.float32
F16 = mybir.dt.float16
I16 = mybir.dt.int16
P = 128

N_CORES = 8
B_FULL, N, D = 16, 2048, 64
B_LOC = B_FULL // N_CORES          # 2 batches per core
NT = N // P                        # 16 tiles of 128 along q and j
QB = 512                           # q-block
NQB = N // QB                      # 4 q-blocks
QTPB = QB // P                     # 4 q-subtiles per q-block
GROUP = 2                          # max S^T slots per PSUM group
SPQ = 2 * NT                       # 32 (j, b) slots per q-block
PV_LAG = 8                        # groups between a drain and its PV use

A_DVE = float(0.125 * 1024.0 / math.log(2.0))
B_DVE = float(15 * 1024) - 0.5

# drain cost model (ns) used for the offline ACT/DVE group assignment
_ACT_EL, _ACT_OH = 1e9 / 1.2e9, 185.0
_DVE_EL, _DVE_OH = 1e9 / 0.96e9, 125.0
_OUT_STAGE_DVE = 1042.0            # recip+mult for both batches of a q-block

_nc_cache = None


def _plan_groups():
    """Greedy static partition of the 32 slots per q-block into drain
    groups: 2-slot groups on ACT (its 185ns PSUM-access overhead amortizes
    over 1024 elements) vs 1-slot groups on DVE.  Whole-group alternation
    pays each group's overhead on one engine only; the asymmetric sizes
    keep both engines' modeled busy time level."""
    act_t = 1283.0                 # ACT one-time exp table load
    dve_t = 0.0
    groups = []                    # (qb, s0, s1, eng)
    slot_group = {}                # (qb, slot) -> group index
    for qb in range(NQB):
        s = 0
        while s < SPQ:
            na = 2 if s + 2 <= SPQ else 1
            if act_t <= dve_t:
                eng, ns = "act", na
                act_t += ns * QB * _ACT_EL + _ACT_OH
            else:
                eng, ns = "dve", 1
                dve_t += QB * _DVE_EL + _DVE_OH
            for i in range(ns):
                slot_group[(qb, s + i)] = len(groups)
            groups.append((qb, s, s + ns, eng))
            s += ns
        dve_t += _OUT_STAGE_DVE
    return groups, slot_group


GROUPS, _SLOT_GROUP = _plan_groups()
NGRP_ALL = len(GROUPS)


def build():
    nc = bacc.Bacc(None, target_bir_lowering=False)
    q_hbm = nc.dram_tensor("queries", [B_LOC, N, D], F32, kind="ExternalInput")
    k_hbm = nc.dram_tensor("keys", [B_LOC, N, D], F32, kind="ExternalInput")
    v_hbm = nc.dram_tensor("values", [B_LOC, N, D], F32, kind="ExternalInput")
    o_hbm = nc.dram_tensor("out", [B_LOC, N, D], F32, kind="ExternalOutput")


    with TileContext(nc) as tc:
        with (
            tc.tile_pool(name="cst", bufs=1) as cst,
            tc.tile_pool(name="stage", bufs=1) as stage,
            tc.tile_pool(name="persist", bufs=1) as persist,
            tc.tile_pool(name="preg", bufs=2) as pregp,
            tc.tile_pool(name="oo", bufs=2) as oop,
            tc.tile_pool(name="sta", bufs=2, space="PSUM") as stpA,
            tc.tile_pool(name="std", bufs=2, space="PSUM") as stpD,
            tc.tile_pool(name="pv", bufs=2, space="PSUM") as pvp,
        ):
            ident = cst.tile([P, P], F16)
            make_identity(nc, ident)

            # PE p-state pre-warm: the Tensor engine reaches full clock only
            # after ~3us of (near-)continuous activity.  Dummy matmuls keep
            # it busy through the load phase so real matmuls start warm.
            warm = pvp.tile([P, P], F32, tag="pv", name="warm")
            for _ in range(18):
                nc.tensor.matmul(warm[:], ident[:], ident[:],
                                 start=True, stop=True)

            # ---- persistent SBUF ----
            # qt/kt: partition = b*64 + d, free = (tile, qcol)
            qt = persist.tile([P, NT, P], F16, tag="qt")
            kt = persist.tile([P, NT, P], F16, tag="kt")
            # V' = [V | ones]: [128 j, b, jt, 65] fp16
            v16 = persist.tile([P, B_LOC, NT, D + 1], F16, tag="v16")

            st32 = {}
            st16i = {}
            for name in ("k", "q"):
                # fp32 staging [q, b, t, d]; one DMA loads both batches
                st32[name] = stage.tile(
                    [P, B_LOC, NT, D], F32, tag=f"{name}32", name=f"{name}32")
                # fp16 staging interleaved as [q, t, (b, d)] so the XBAR
                # transpose's 128-wide column tiles map one output tile each
                st16i[name] = stage.tile(
                    [P, NT, B_LOC, D], F16, tag=f"{name}16", name=f"{name}16")
            vs32 = stage.tile([P, B_LOC, NT, D], F32, tag="vs32")

            nc.gpsimd.memset(v16[:, :, :, D:D + 1], 1.0)

            # ---- loads on SP HWDGE, both batches per DMA, K first ----
            def load(name, t0, t1):
                hbm = {"k": k_hbm, "q": q_hbm, "v": v_hbm}[name]
                src = hbm.rearrange("b (p t) d -> p b t d", p=P)[:, :, t0:t1]
                dst = vs32 if name == "v" else st32[name]
                nc.sync.dma_start(dst[:, :, t0:t1], src)

            # ---- casts: first Q/K chunks on the still-idle DVE/ACT so the
            # first transposes fire early; the rest on Pool in load order
            def cast(name, t0, t1, eng=None):
                if name == "v":
                    nc.gpsimd.tensor_copy(v16[:, :, t0:t1, 0:D],
                                          vs32[:, :, t0:t1])
                else:
                    (eng or nc.gpsimd.tensor_copy)(
                        st16i[name][:, t0:t1, :, :],
                        st32[name][:, :, t0:t1].rearrange("p b t d -> p t b d"))

            # ---- XBAR DMA transposes: st16i [128 q, t, (b,d)] -> qt/kt
            # [128 rows (b*64+d), t, 128 q];  out[p, t, c] = in[c, t*128+p]
            def tp(name, t0, t1):
                dst = kt if name == "k" else qt
                nc.sync.dma_start(dst[:, t0:t1, :],
                                  st16i[name][:, t0:t1, :, :], transpose=True)

            # PE transpose for the startup-critical early tiles: avoids a
            # DMA hop (load -> cast -> PE tp -> engine drain is all
            # engine-chained, ~2.5us faster than a DMA transpose hop)
            def pe_tp(name, t0, drain):
                dst = kt if name == "k" else qt
                tp_ps = stpA.tile([P, 4, P], F16, tag="st",
                                 name=f"tp_{name}{t0}")
                for t in range(t0, t0 + 4):
                    nc.tensor.transpose(tp_ps[:, t - t0, :], st16i[name][:, t],
                                        ident[:])
                drain(dst[:, t0:t0 + 4, :], tp_ps[:])

            # All DMAs ride the single SP queue (cross-queue DMA pairs get
            # sem-chained by the tile scheduler at ~1.5us per switch).
            load("k", 0, 4)
            load("q", 0, 4)
            cast("k", 0, 4, nc.vector.tensor_copy)
            cast("q", 0, 4, nc.scalar.copy)
            pe_tp("k", 0, nc.vector.tensor_copy)
            pe_tp("q", 0, nc.scalar.copy)
            # Loads first (the SP DMA queue is FIFO -- a cast-gated transpose
            # at the SEQ head would stall later load issues)
            load("k", 4, 8)
            load("k", 8, 12)
            load("k", 12, 16)
            load("v", 0, 8)
            load("q", 4, 10)
            load("v", 8, 16)
            load("q", 10, 16)
            for args in (("k", 4, 8), ("k", 8, 12), ("k", 12, 16),
                         ("v", 0, 8), ("q", 4, 10), ("v", 8, 16),
                         ("q", 10, 16)):
                cast(*args)
            # All of K via PE transpose (kt is needed tile-by-tile from the
            # very first groups; the DMA-transpose FIFO lands tiles 8:16
            # ~2.5us too late).  Q beyond the first block has lots of slack
            # and rides the DMA XBAR instead, keeping drain engines free.
            pe_tp("k", 4, nc.vector.tensor_copy)
            pe_tp("k", 8, nc.vector.tensor_copy)
            pe_tp("k", 12, nc.scalar.copy)
            tp("q", 4, 10)
            tp("q", 10, 16)

            # ---- flat software pipeline over all 64 groups ----
            preg = {}
            preg_i = {}
            pv = {}
            next_pv = 0

            # slot -> (j-tile, batch).  The last q-block runs batch-major so
            # batch 0's normalize+store happens mid-block and only batch 1's
            # drain->PV->normalize->store chain trails the final drain.
            def jb_of(qb, s):
                if qb == NQB - 1:
                    return s % NT, s // NT
                return s >> 1, s & 1

            # v16 chunk readiness as a conservative "earliest group" gate
            def v_gate(step):
                qb, rem = divmod(step, 2 * NT)
                k, b = jb_of(qb, rem)
                if k < 8:
                    return 12 + 2 * b
                return 20 + 2 * b

            def emit_group(g):
                qb, s0, s1, eng = GROUPS[g]
                ns = s1 - s0
                if s0 == 0:
                    preg[qb] = pregp.tile([P, SPQ, QB], F16, tag="preg",
                                          name=f"preg{qb}")
                    preg_i[qb] = preg[qb][:].bitcast(I16)
                pool, w = (stpA, GROUP) if eng == "act" else (stpD, 1)
                st_t = pool.tile([P, w, QB], F32, tag="st",
                                 name=f"st{qb}_{s0}")
                for i in range(ns):
                    s = s0 + i
                    j, b = jb_of(qb, s)
                    rows = slice(b * D, (b + 1) * D)
                    nc.tensor.matmul(
                        st_t[:, i, :],
                        kt[rows, j, :],
                        qt[rows, qb * QTPB:(qb + 1) * QTPB, :],
                        start=True, stop=True,
                    )
                sl = slice(s0, s1)
                if eng == "act":
                    nc.scalar.activation(
                        preg[qb][:, sl, :], st_t[:, 0:ns, :],
                        mybir.ActivationFunctionType.Exp, scale=0.125,
                    )
                else:
                    nc.vector.tensor_scalar(
                        out=preg_i[qb][:, sl, :], in0=st_t[:, 0:ns, :],
                        scalar1=A_DVE, scalar2=B_DVE,
                        op0=mybir.AluOpType.mult, op1=mybir.AluOpType.add,
                    )

            def emit_pv_half(step):
                qb, rem = divmod(step, 2 * NT)
                k, b = jb_of(qb, rem)
                if k == 0:
                    pv[(qb, b)] = pvp.tile([P, QTPB, P], F32, tag="pv",
                                           name=f"pv{qb}_{b}")
                for t in range(QTPB):
                    # PSUM zeroing granularity is the full 2KB bank: only
                    # each bank's first matmul may set start=True; later
                    # chains' first writes overwrite via the bank-wide
                    # pending-zero (PE runs in program order).
                    nc.tensor.matmul(
                        pv[(qb, b)][:, t, 0:D + 1],
                        preg[qb][:, rem, t * P:(t + 1) * P],
                        v16[:, b, k, :],
                        start=(k == 0 and t == 0),
                        stop=(k == NT - 1),
                        skip_group_check=True,
                    )
                if k == NT - 1:
                    emit_out_stage(qb, b)

            def emit_out_stage(qb, b):
                rec = oop.tile([P, QTPB, 1], F32, tag="rec",
                               name=f"rec{qb}_{b}")
                nc.vector.reciprocal(rec[:], pv[(qb, b)][:, :, D:D + 1])
                o_out = oop.tile([P, QTPB, D], F32, tag="oo",
                                 name=f"oo{qb}_{b}")
                nc.vector.tensor_tensor(
                    o_out[:], pv[(qb, b)][:, :, 0:D],
                    rec[:].to_broadcast((P, QTPB, D)),
                    mybir.AluOpType.mult,
                )
                o_dst = o_hbm[b, :, :].rearrange("(p t) d -> p t d", p=P)
                nc.sync.dma_start(
                    o_dst[:, qb * QTPB:(qb + 1) * QTPB, :], o_out[:])

            def pv_req_group(step):
                qb, rem = divmod(step, 2 * NT)
                return _SLOT_GROUP[(qb, rem)]

            NPV = NQB * NT * 2
            for g in range(NGRP_ALL):
                # lag tapers near the end so no PV backlog trails the last
                # drain (the tail chain is drain -> PV -> recip/mult -> store)
                lag = min(PV_LAG, max(1, NGRP_ALL - 3 - g))
                while (next_pv < NPV
                       and pv_req_group(next_pv) <= g - 1 - lag
                       and v_gate(next_pv) <= g):
                    emit_pv_half(next_pv)
                    next_pv += 1
                emit_group(g)
            while next_pv < NPV:
                emit_pv_half(next_pv)
                next_pv += 1

    nc.compile()
    return nc


def get_nc():
    global _nc_cache
    if _nc_cache is None:
        _nc_cache = build()
    return _nc_cache


def kernel(queries: np.ndarray, keys: np.ndarray, values: np.ndarray) -> np.ndarray:
    from concourse.bass_utils import run_bass_kernel_spmd

    queries = np.ascontiguousarray(np.asarray(queries, dtype=np.float32))
    keys = np.ascontiguousarray(np.asarray(keys, dtype=np.float32))
    values = np.ascontiguousarray(np.asarray(values, dtype=np.float32))

    nc = get_nc()
    in_maps = []
    for c in range(N_CORES):
        sl = slice(c * B_LOC, (c + 1) * B_LOC)
        in_maps.append({
            "queries": queries[sl],
            "keys": keys[sl],
            "values": values[sl],
        })
    res = run_bass_kernel_spmd(nc, in_maps, core_ids=list(range(N_CORES)))
    return np.concatenate([r["out"] for r in res.results], axis=0)


if __name__ == "__main__":
    rng = np.random.default_rng(0)
    q = rng.standard_normal((B_FULL, N, D), dtype=np.float32)
    k = rng.standard_normal((B_FULL, N, D), dtype=np.float32)
    v = rng.standard_normal((B_FULL, N, D), dtype=np.float32)
    o = kernel(queries=q, keys=k, values=v)
    s = q @ k.transpose(0, 2, 1) / np.sqrt(D)
    w = np.exp(s - s.max(-1, keepdims=True))
    w /= w.sum(-1, keepdims=True)
    ref = w @ v
    err = np.abs(o - ref).max() / np.abs(ref).max()
    print("rel err:", err)


# revision 70
# speedup vs baseline: 1.0050x; 1.0050x over previous
"""Trainium2 Bass kernel: batched dense attention (v4).

Full inputs: queries/keys/values [16, 2048, 64] fp32.
Shards batch dim across 8 NeuronCores (2 batches per core).

Per-core pipeline (batches A=0, B=1 local):
  HBM loads use the "(p t)" row split (row n = p*16 + t) so every DMA
  descriptor moves >=1KB contiguous (no <512B 2x latency penalty).
  Q/K: HBM fp32 -> staged SBUF -> cast fp16 (interleaved [q, t, (b,d)])
  -> transpose into qt/kt [(b*64+d), t, q].  K (needed tile-by-tile from
  the first groups) via PE transposes drained by DVE/ACT; Q tiles 4:16
  (lots of slack) via XBAR DMA transpose, keeping drain engines free.
  S^T[j, q] = K . Q    (PE fp16, fp32 PSUM, 1-2 slot groups, 6-bank ring)
  P^T = exp(S^T / 8) in fp16; WHOLE drain groups alternate between:
     ACT: exact exp (scale=0.125 fused), 2-slot groups (amortizes its
          185ns PSUM/SBUF access overhead over 1024 elements)
     DVE: Schraudolph bit-trick exp (int16 mult+add == fp16 exp bits),
          1-slot groups
  via a greedy offline plan that levels both engines' modeled busy time.
  O[q, d] = sum_j P^T[j, q] V'[j, d]  (PE: P^T stationary, V'=[V|1]
  moving; 65-col matmuls; softmax sums ride the ones column)
  out = O[:, 0:64] * (1 / O[:, 64])   (DVE reciprocal + mult from PSUM)

Scheduling notes (TimelineSim cost model):
  - The pipeline is PE-throughput-bound (~321 ns/slot: 213 QK + 108 PV);
    drain engines run ~94% of PE busy.  Dummy matmuls pre-warm the PE
    p-state ramp (full clock needs ~3us of activity) during loads.
  - Every DMA rides the single SP HWDGE queue in FIFO order; cross-queue
    DMA pairs get sem-chained by the tile scheduler (~1.5us per switch).
  - The last q-block runs batch-major so only batch 1's drain -> PV ->
    normalize -> store chain (~4.2us) trails the final drain.
  - fp8e4m3 DoubleRow QK was tried and rejected: Q/K quantization costs
    ~10% output error (softmax amplifies score noise) vs the 2e-2 budget.
  - PSUM: 2x2-bank ACT ring + 2x1-bank DVE ring + 2 PV banks = 8 banks.
"""

import sys
for _p in ("/opt/trn_rl_repo", "/root/.axon_site/_ro/trn_rl_repo"):
    if _p not in sys.path:
        sys.path.insert(0, _p)

import math
import numpy as np

import concourse.bass as bass
import concourse.mybir as mybir
from concourse import bacc
from concourse.tile import TileContext
from concourse.masks import make_identity

F32 = mybir.dt.float32
F16 = mybir.dt.float16
I16 = mybir.dt.int16
P = 128

N_CORES = 8
B_FULL, N, D = 16, 2048, 64
B_LOC = B_FULL // N_CORES          # 2 batches per core
NT = N // P                        # 16 tiles of 128 along q and j
QB = 512                           # q-block
NQB = N // QB                      # 4 q-blocks
QTPB = QB // P                     # 4 q-subtiles per q-block
GROUP = 2                          # max S^T slots per PSUM group
SPQ = 2 * NT                       # 32 (j, b) slots per q-block
PV_LAG = 12                        # groups between a drain and its PV use

A_DVE = float(0.125 * 1024.0 / math.log(2.0))
B_DVE = float(15 * 1024) - 0.5

# drain cost model (ns) used for the offline ACT/DVE group assignment
_ACT_EL, _ACT_OH = 1e9 / 1.2e9, 185.0
_DVE_EL, _DVE_OH = 1e9 / 0.96e9, 125.0
_OUT_STAGE_DVE = 1042.0            # recip+mult for both batches of a q-block

_nc_cache = None


def _plan_groups():
    """Greedy static partition of the 32 slots per q-block into drain
    groups: 2-slot groups on ACT (its 185ns PSUM-access overhead amortizes
    over 1024 elements) vs 1-slot groups on DVE.  Whole-group alternation
    pays each group's overhead on one engine only; the asymmetric sizes
    keep both engines' modeled busy time level."""
    act_t = 1283.0                 # ACT one-time exp table load
    dve_t = 0.0
    groups = []                    # (qb, s0, s1, eng)
    slot_group = {}                # (qb, slot) -> group index
    for qb in range(NQB):
        s = 0
        while s < SPQ:
            na = 2 if s + 2 <= SPQ else 1
            if act_t <= dve_t:
                eng, ns = "act", na
                act_t += ns * QB * _ACT_EL + _ACT_OH
            else:
                eng, ns = "dve", 1
                dve_t += QB * _DVE_EL + _DVE_OH
            for i in range(ns):
                slot_group[(qb, s + i)] = len(groups)
            groups.append((qb, s, s + ns, eng))
            s += ns
        dve_t += _OUT_STAGE_DVE
    return groups, slot_group


GROUPS, _SLOT_GROUP = _plan_groups()
NGRP_ALL = len(GROUPS)


def build():
    nc = bacc.Bacc(None, target_bir_lowering=False)
    q_hbm = nc.dram_tensor("queries", [B_LOC, N, D], F32, kind="ExternalInput")
    k_hbm = nc.dram_tensor("keys", [B_LOC, N, D], F32, kind="ExternalInput")
    v_hbm = nc.dram_tensor("values", [B_LOC, N, D], F32, kind="ExternalInput")
    o_hbm = nc.dram_tensor("out", [B_LOC, N, D], F32, kind="ExternalOutput")


    with TileContext(nc) as tc:
        with (
            tc.tile_pool(name="cst", bufs=1) as cst,
            tc.tile_pool(name="stage", bufs=1) as stage,
            tc.tile_pool(name="persist", bufs=1) as persist,
            tc.tile_pool(name="preg", bufs=2) as pregp,
            tc.tile_pool(name="oo", bufs=2) as oop,
            tc.tile_pool(name="sta", bufs=2, space="PSUM") as stpA,
            tc.tile_pool(name="std", bufs=2, space="PSUM") as stpD,
            tc.tile_pool(name="pv", bufs=2, space="PSUM") as pvp,
        ):
            ident = cst.tile([P, P], F16)
            make_identity(nc, ident)

            # PE p-state pre-warm: the Tensor engine reaches full clock only
            # after ~3us of (near-)continuous activity.  Dummy matmuls keep
            # it busy through the load phase so real matmuls start warm.
            warm = pvp.tile([P, P], F32, tag="pv", name="warm")
            for _ in range(18):
                nc.tensor.matmul(warm[:], ident[:], ident[:],
                                 start=True, stop=True)

            # ---- persistent SBUF ----
            # qt/kt: partition = b*64 + d, free = (tile, qcol)
            qt = persist.tile([P, NT, P], F16, tag="qt")
            kt = persist.tile([P, NT, P], F16, tag="kt")
            # V' = [V | ones]: [128 j, b, jt, 65] fp16
            v16 = persist.tile([P, B_LOC, NT, D + 1], F16, tag="v16")

            st32 = {}
            st16i = {}
            for name in ("k", "q"):
                # fp32 staging [q, b, t, d]; one DMA loads both batches
                st32[name] = stage.tile(
                    [P, B_LOC, NT, D], F32, tag=f"{name}32", name=f"{name}32")
                # fp16 staging interleaved as [q, t, (b, d)] so the XBAR
                # transpose's 128-wide column tiles map one output tile each
                st16i[name] = stage.tile(
                    [P, NT, B_LOC, D], F16, tag=f"{name}16", name=f"{name}16")
            vs32 = stage.tile([P, B_LOC, NT, D], F32, tag="vs32")

            nc.gpsimd.memset(v16[:, :, :, D:D + 1], 1.0)

            # ---- loads on SP HWDGE, both batches per DMA, K first ----
            def load(name, t0, t1):
                hbm = {"k": k_hbm, "q": q_hbm, "v": v_hbm}[name]
                src = hbm.rearrange("b (p t) d -> p b t d", p=P)[:, :, t0:t1]
                dst = vs32 if name == "v" else st32[name]
                nc.sync.dma_start(dst[:, :, t0:t1], src)

            # ---- casts: first Q/K chunks on the still-idle DVE/ACT so the
            # first transposes fire early; the rest on Pool in load order
            def cast(name, t0, t1, eng=None):
                if name == "v":
                    nc.gpsimd.tensor_copy(v16[:, :, t0:t1, 0:D],
                                          vs32[:, :, t0:t1])
                else:
                    (eng or nc.gpsimd.tensor_copy)(
                        st16i[name][:, t0:t1, :, :],
                        st32[name][:, :, t0:t1].rearrange("p b t d -> p t b d"))

            # ---- XBAR DMA transposes: st16i [128 q, t, (b,d)] -> qt/kt
            # [128 rows (b*64+d), t, 128 q];  out[p, t, c] = in[c, t*128+p]
            def tp(name, t0, t1):
                dst = kt if name == "k" else qt
                nc.sync.dma_start(dst[:, t0:t1, :],
                                  st16i[name][:, t0:t1, :, :], transpose=True)

            # PE transpose for the startup-critical early tiles: avoids a
            # DMA hop (load -> cast -> PE tp -> engine drain is all
            # engine-chained, ~2.5us faster than a DMA transpose hop)
            def pe_tp(name, t0, drain):
                dst = kt if name == "k" else qt
                tp_ps = stpA.tile([P, 4, P], F16, tag="st",
                                 name=f"tp_{name}{t0}")
                for t in range(t0, t0 + 4):
                    nc.tensor.transpose(tp_ps[:, t - t0, :], st16i[name][:, t],
                                        ident[:])
                drain(dst[:, t0:t0 + 4, :], tp_ps[:])

            # All DMAs ride the single SP queue (cross-queue DMA pairs get
            # sem-chained by the tile scheduler at ~1.5us per switch).
            # q first: its chain (ACT copy-cast, 612ns) is longer than K's
            # (DVE 2x cast, 327ns); the first QK group needs both.
            load("q", 0, 4)
            load("k", 0, 4)
            cast("q", 0, 4, nc.scalar.copy)
            cast("k", 0, 4, nc.vector.tensor_copy)
            pe_tp("q", 0, nc.scalar.copy)
            pe_tp("k", 0, nc.vector.tensor_copy)
            # Loads first (the SP DMA queue is FIFO -- a cast-gated transpose
            # at the SEQ head would stall later load issues)
            load("k", 4, 8)
            load("k", 8, 12)
            load("k", 12, 16)
            load("v", 0, 8)
            load("q", 4, 10)
            load("v", 8, 16)
            load("q", 10, 16)
            for args in (("k", 4, 8), ("k", 8, 12), ("k", 12, 16),
                         ("v", 0, 8), ("q", 4, 10), ("v", 8, 16),
                         ("q", 10, 16)):
                cast(*args)
            # All of K via PE transpose (kt is needed tile-by-tile from the
            # very first groups; the DMA-transpose FIFO lands tiles 8:16
            # ~2.5us too late).  Q beyond the first block has lots of slack
            # and rides the DMA XBAR instead, keeping drain engines free.
            pe_tp("k", 4, nc.vector.tensor_copy)
            pe_tp("k", 8, nc.vector.tensor_copy)
            pe_tp("k", 12, nc.scalar.copy)
            tp("q", 4, 10)
            tp("q", 10, 16)

            # ---- flat software pipeline over all 64 groups ----
            preg = {}
            preg_i = {}
            pv = {}
            next_pv = 0

            # slot -> (j-tile, batch).  The last q-block runs batch-major so
            # batch 0's normalize+store happens mid-block and only batch 1's
            # drain->PV->normalize->store chain trails the final drain.
            def jb_of(qb, s):
                if qb == NQB - 1:
                    return s % NT, s // NT
                return s >> 1, s & 1

            # v16 chunk readiness as a conservative "earliest group" gate
            def v_gate(step):
                qb, rem = divmod(step, 2 * NT)
                k, b = jb_of(qb, rem)
                if k < 8:
                    return 12 + 2 * b
                return 20 + 2 * b

            def emit_group(g):
                qb, s0, s1, eng = GROUPS[g]
                ns = s1 - s0
                if s0 == 0:
                    preg[qb] = pregp.tile([P, SPQ, QB], F16, tag="preg",
                                          name=f"preg{qb}")
                    preg_i[qb] = preg[qb][:].bitcast(I16)
                pool, w = (stpA, GROUP) if eng == "act" else (stpD, 1)
                st_t = pool.tile([P, w, QB], F32, tag="st",
                                 name=f"st{qb}_{s0}")
                for i in range(ns):
                    s = s0 + i
                    j, b = jb_of(qb, s)
                    rows = slice(b * D, (b + 1) * D)
                    nc.tensor.matmul(
                        st_t[:, i, :],
                        kt[rows, j, :],
                        qt[rows, qb * QTPB:(qb + 1) * QTPB, :],
                        start=True, stop=True,
                    )
                sl = slice(s0, s1)
                if eng == "act":
                    nc.scalar.activation(
                        preg[qb][:, sl, :], st_t[:, 0:ns, :],
                        mybir.ActivationFunctionType.Exp, scale=0.125,
                    )
                else:
                    nc.vector.tensor_scalar(
                        out=preg_i[qb][:, sl, :], in0=st_t[:, 0:ns, :],
                        scalar1=A_DVE, scalar2=B_DVE,
                        op0=mybir.AluOpType.mult, op1=mybir.AluOpType.add,
                    )

            def emit_pv_half(step):
                qb, rem = divmod(step, 2 * NT)
                k, b = jb_of(qb, rem)
                if k == 0:
                    pv[(qb, b)] = pvp.tile([P, QTPB, P], F32, tag="pv",
                                           name=f"pv{qb}_{b}")
                for t in range(QTPB):
                    # PSUM zeroing granularity is the full 2KB bank: only
                    # each bank's first matmul may set start=True; later
                    # chains' first writes overwrite via the bank-wide
                    # pending-zero (PE runs in program order).
                    nc.tensor.matmul(
                        pv[(qb, b)][:, t, 0:D + 1],
                        preg[qb][:, rem, t * P:(t + 1) * P],
                        v16[:, b, k, :],
                        start=(k == 0 and t == 0),
                        stop=(k == NT - 1),
                        skip_group_check=True,
                    )
                if k == NT - 1:
                    emit_out_stage(qb, b)

            def emit_out_stage(qb, b):
                rec = oop.tile([P, QTPB, 1], F32, tag="rec",
                               name=f"rec{qb}_{b}")
                nc.vector.reciprocal(rec[:], pv[(qb, b)][:, :, D:D + 1])
                o_out = oop.tile([P, QTPB, D], F32, tag="oo",
                                 name=f"oo{qb}_{b}")
                nc.vector.tensor_tensor(
                    o_out[:], pv[(qb, b)][:, :, 0:D],
                    rec[:].to_broadcast((P, QTPB, D)),
                    mybir.AluOpType.mult,
                )
                o_dst = o_hbm[b, :, :].rearrange("(p t) d -> p t d", p=P)
                nc.sync.dma_start(
                    o_dst[:, qb * QTPB:(qb + 1) * QTPB, :], o_out[:])

            def pv_req_group(step):
                qb, rem = divmod(step, 2 * NT)
                return _SLOT_GROUP[(qb, rem)]

            NPV = NQB * NT * 2
            for g in range(NGRP_ALL):
                # lag tapers near the end so no PV backlog trails the last
                # drain (the tail chain is drain -> PV -> recip/mult -> store)
                lag = min(PV_LAG, max(1, NGRP_ALL - 3 - g))
                while (next_pv < NPV
                       and pv_req_group(next_pv) <= g - 1 - lag
                       and v_gate(next_pv) <= g):
                    emit_pv_half(next_pv)
                    next_pv += 1
                emit_group(g)
            while next_pv < NPV:
                emit_pv_half(next_pv)
                next_pv += 1

    nc.compile()
    return nc


def get_nc():
    global _nc_cache
    if _nc_cache is None:
        _nc_cache = build()
    return _nc_cache


def kernel(queries: np.ndarray, keys: np.ndarray, values: np.ndarray) -> np.ndarray:
    from concourse.bass_utils import run_bass_kernel_spmd

    queries = np.ascontiguousarray(np.asarray(queries, dtype=np.float32))
    keys = np.ascontiguousarray(np.asarray(keys, dtype=np.float32))
    values = np.ascontiguousarray(np.asarray(values, dtype=np.float32))

    nc = get_nc()
    in_maps = []
    for c in range(N_CORES):
        sl = slice(c * B_LOC, (c + 1) * B_LOC)
        in_maps.append({
            "queries": queries[sl],
            "keys": keys[sl],
            "values": values[sl],
        })
    res = run_bass_kernel_spmd(nc, in_maps, core_ids=list(range(N_CORES)))
    return np.concatenate([r["out"] for r in res.results], axis=0)


if __name__ == "__main__":
    rng = np.random.default_rng(0)
    q = rng.standard_normal((B_FULL, N, D), dtype=np.float32)
    k = rng.standard_normal((B_FULL, N, D), dtype=np.float32)
    v = rng.standard_normal((B_FULL, N, D), dtype=np.float32)
    o = kernel(queries=q, keys=k, values=v)
    s = q @ k.transpose(0, 2, 1) / np.sqrt(D)
    w = np.exp(s - s.max(-1, keepdims=True))
    w /= w.sum(-1, keepdims=True)
    ref = w @ v
    err = np.abs(o - ref).max() / np.abs(ref).max()
    print("rel err:", err)
